# revision 1
# baseline (speedup 1.0000x reference)
"""AdaptiveCurvatureLoss on 8 TRN2 NeuronCores — bitonic-sort kNN variant.

The kNN density of a 1-D point set needs only the sorted order: each point's
two nearest neighbours lie within +-2 positions in sorted order.  So instead
of the O(N^2) pairwise matrix, every core sorts the full x (16384 values as a
[128, 128] tile) with a bitonic network:
  - compare-exchange stages along the free dim (pairs at distance j) as
    tensor_tensor min/max over strided views,
  - descending blocks handled by per-partition sign flips (host-supplied
    masks) for k >= 128, and by separate asc/desc views for k <= 64,
  - pair distances >= 128 via PE transpose (work in transposed index space).
Then neighbour diffs + a 4-candidate window give exact f32 densities.
The MLP / second-derivative / MSE parts stay row-sharded across the 8 cores
as before; host combines partial sums (scalar epilogue only).
"""

import sys

sys.path.insert(0, "/opt/trn_rl_repo")

import numpy as np

import concourse.mybir as mybir
from concourse import bacc
from concourse.bass_utils import run_bass_kernel_spmd
from concourse.tile import TileContext

N = 16384
NCORES = 8
SHARD = N // NCORES          # 2048
P = 128
W = 128                      # sort grid: [128 partitions, 128 free]
NCH = SHARD // P             # 16
H = 64
EPS = 1e-8
BIG = 1e30
F32 = mybir.dt.float32
ALU = mybir.AluOpType
ACTF = mybir.ActivationFunctionType

SGN_KS = [128 << t for t in range(7)]  # 128..8192


def _build():
    nc = bacc.Bacc()
    x_full = nc.declare_dram_parameter("x_full", [N], F32, isOutput=False)
    t_sh = nc.declare_dram_parameter("t_shard", [P, NCH], F32, isOutput=False)
    w1 = nc.declare_dram_parameter("w1", [H], F32, isOutput=False)
    b1 = nc.declare_dram_parameter("b1", [H], F32, isOutput=False)
    w2 = nc.declare_dram_parameter("w2", [H], F32, isOutput=False)
    b2 = nc.declare_dram_parameter("b2", [1], F32, isOutput=False)
    c2n = nc.declare_dram_parameter("c2n", [H], F32, isOutput=False)  # 2*w1^2*w2
    xf = nc.declare_dram_parameter("x_flat", [1, 2 * SHARD], F32, isOutput=False)  # [x | ones]
    sgn = nc.declare_dram_parameter("signs", [P, len(SGN_KS)], F32, isOutput=False)
    idn = nc.declare_dram_parameter("ident", [P, P], F32, isOutput=False)
    shu = nc.declare_dram_parameter("shiftu", [P, P], F32, isOutput=False)
    shd = nc.declare_dram_parameter("shiftd", [P, P], F32, isOutput=False)
    out = nc.declare_dram_parameter("out", [P, W + 2], F32, isOutput=True)

    with TileContext(nc) as tc:
        with (
            tc.tile_pool(name="sp", bufs=1) as sp,
            tc.tile_pool(name="ps", bufs=2, space="PSUM") as ps,
            tc.tile_pool(name="ups", bufs=2, space="PSUM") as upsp,
        ):
            # ---- loads ----
            sortA = sp.tile([P, W], F32)
            nc.sync.dma_start(sortA[:, :], x_full.ap().rearrange("(p f) -> p f", p=P))
            sortB = sp.tile([P, W], F32)
            signs = sp.tile([P, len(SGN_KS)], F32)
            nc.sync.dma_start(signs[:, :], sgn[:, :])
            ident = sp.tile([P, P], F32)
            nc.sync.dma_start(ident[:, :], idn[:, :])
            # non-sort-critical loads go on the gpsimd DMA queue so they don't
            # serialize behind the sort's sync-queue DMAs
            shiftu = sp.tile([P, P], F32)
            nc.gpsimd.dma_start(shiftu[:, :], shu[:, :])
            shiftd = sp.tile([P, P], F32)
            nc.gpsimd.dma_start(shiftd[:, :], shd[:, :])
            tsh = sp.tile([P, NCH], F32)
            nc.gpsimd.dma_start(tsh[:, :], t_sh[:, :])
            w1r = sp.tile([P, H], F32)
            nc.gpsimd.dma_start(w1r[:, :], w1.ap().partition_broadcast(P))
            b1r = sp.tile([P, H], F32)
            nc.gpsimd.dma_start(b1r[:, :], b1.ap().partition_broadcast(P))
            w2r = sp.tile([P, H], F32)
            nc.gpsimd.dma_start(w2r[:, :], w2.ap().partition_broadcast(P))
            c2r = sp.tile([P, H], F32)
            nc.gpsimd.dma_start(c2r[:, :], c2n.ap().partition_broadcast(P))
            b2s = sp.tile([P, 1], F32)
            nc.gpsimd.dma_start(b2s[:, :], b2.ap().partition_broadcast(P))
            xfs = sp.tile([1, 2 * SHARD], F32)
            nc.gpsimd.dma_start(xfs[:, :], xf[:, :])
            out_sb = sp.tile([P, W + 2], F32)
            # BIG row for the auxU boundary, prepared up front
            auxU = sp.tile([P, 2], F32)
            bigc = sp.tile([P, 2], F32)
            nc.vector.memset(bigc[:, :], BIG)
            nc.gpsimd.dma_start(auxU[P - 1 : P, 0:2], bigc[0:1, 0:2])

            # ---- MLP tiles + chunked emitter (interleaved into the sort) ----
            u = sp.tile([P, NCH, H], F32)
            th = sp.tile([P, NCH, H], F32)
            g = sp.tile([P, NCH, H], F32)
            pred = sp.tile([P, NCH], F32)
            d2t = sp.tile([P, NCH], F32)
            mlp_next = [0]
            mlp_dve_pending = []

            def emit_mlp_pe_act(c):
                ups = upsp.tile([P, H], F32, tag="ups")
                cs = slice(c * P, (c + 1) * P)
                cso = slice(SHARD + c * P, SHARD + (c + 1) * P)
                nc.tensor.matmul(ups[:, :], xfs[0:1, cs], w1r[0:1, :], start=True, stop=False)
                nc.tensor.matmul(ups[:, :], xfs[0:1, cso], b1r[0:1, :], start=False, stop=True)
                nc.scalar.activation(th[:, c, :], ups[:, :], ACTF.Tanh)
                nc.scalar.activation(u[:, c, :], th[:, c, :], ACTF.Square)

            def emit_mlp_dve(c):
                nc.vector.scalar_tensor_tensor(
                    g[:, c, :], u[:, c, :], 1.0, th[:, c, :], op0=ALU.subtract, op1=ALU.mult
                )
                nc.vector.tensor_tensor(u[:, c, :], th[:, c, :], w2r[:, :], op=ALU.mult)
                nc.vector.tensor_reduce(
                    pred[:, c : c + 1], u[:, c, :], axis=mybir.AxisListType.X, op=ALU.add
                )
                nc.vector.tensor_tensor(u[:, c, :], g[:, c, :], c2r[:, :], op=ALU.mult)
                nc.vector.tensor_reduce(
                    d2t[:, c : c + 1], u[:, c, :], axis=mybir.AxisListType.X, op=ALU.add
                )

            def mlp_window():
                # DVE ops for the chunk issued one window ago (deps long ready),
                # then PE/ACT for the next chunk - fills the transpose stall.
                if mlp_dve_pending:
                    emit_mlp_dve(mlp_dve_pending.pop(0))
                c = mlp_next[0]
                if c < NCH:
                    mlp_next[0] = c + 1
                    emit_mlp_pe_act(c)
                    mlp_dve_pending.append(c)

            # ---- bitonic sort ----
            def lo_hi_views(t, k, j):
                """(lo, hi, is_asc) view pairs of a [P, W] tile t for one stage."""
                if k >= W:
                    v = t[:, :].rearrange("p (c s) -> p c s", s=2 * j)
                    return [(v[:, :, 0:j], v[:, :, j : 2 * j], True)]
                v = t[:, :].rearrange("p (b r) -> p b r", r=2 * k)
                asc = v[:, :, 0:k].rearrange("p b (c s) -> p b c s", s=2 * j)
                desc = v[:, :, k : 2 * k].rearrange("p b (c s) -> p b c s", s=2 * j)
                return [
                    (asc[:, :, :, 0:j], asc[:, :, :, j : 2 * j], True),
                    (desc[:, :, :, 0:j], desc[:, :, :, j : 2 * j], False),
                ]

            # A phase's opening negate rides the post-transpose PSUM->SBUF
            # copy (ACT scale); its closing negate is a cheap in-place DVE
            # tensor_scalar.  Transposes use the fast PE is_transpose path.
            cur, alt = sortA, sortB

            def do_stage(k, j):
                nonlocal cur, alt
                for lo, hi, is_asc in lo_hi_views(cur, k, j):
                    alo, ahi, _ = lo_hi_views(alt, k, j)[0 if is_asc else 1]
                    nc.vector.tensor_tensor(alo, lo, hi, op=ALU.min if is_asc else ALU.max)
                    nc.vector.tensor_tensor(ahi, lo, hi, op=ALU.max if is_asc else ALU.min)
                cur, alt = alt, cur

            def do_transpose(scale_col):
                nonlocal cur, alt
                pt = ps.tile([P, W], F32, tag="tpsum", bufs=3)
                nc.tensor.transpose(pt[:, :], cur[:, :], ident[:, :])
                if scale_col is not None:
                    nc.scalar.mul(alt[:, :], pt[:, :], signs[:, scale_col : scale_col + 1])
                else:
                    nc.scalar.copy(alt[:, :], pt[:, :])
                cur, alt = alt, cur

            for t in range(1, 15):
                k = 1 << t
                js = [k >> s for s in range(1, 20) if (k >> s) >= 1]
                if k <= 64:
                    for j in js:
                        do_stage(k, j)
                    continue
                cross = [j for j in js if j >= W]
                if cross:
                    do_transpose(None)
                    mlp_window()
                    for j in cross:
                        do_stage(min(k // W, W), j // W)
                    do_transpose(SGN_KS.index(k) if k < N else None)
                    mlp_window()
                else:
                    # k == 128: opening negate as a standalone ACT scaled copy
                    nc.scalar.mul(alt[:, :], cur[:, :], signs[:, 0:1])
                    cur, alt = alt, cur
                    mlp_window()
                for j in js:
                    if j < W:
                        do_stage(W, j)
                if 128 <= k < N:
                    # closing un-negate, in place on the DVE
                    col = SGN_KS.index(k)
                    nc.vector.tensor_scalar(
                        cur[:, :], cur[:, :], signs[:, col : col + 1], None, op0=ALU.mult
                    )

            while mlp_dve_pending or mlp_next[0] < NCH:
                mlp_window()
            e = sp.tile([P, NCH], F32)
            esq = sp.tile([P, NCH], F32)
            nc.vector.scalar_tensor_tensor(
                e[:, :], pred[:, :], b2s[:, 0:1], tsh[:, :], op0=ALU.add, op1=ALU.subtract
            )
            nc.scalar.activation(
                esq[:, :], e[:, :], ACTF.Square, accum_out=out_sb[:, W : W + 1]
            )
            nc.scalar.activation(
                esq[:, :], d2t[:, :], ACTF.Square, accum_out=out_sb[:, W + 1 : W + 2]
            )

            s = cur  # sorted ascending, idx = p*W + f

            # ---- neighbour diffs + 4-candidate window ----
            # Row-boundary values via PE shift-matrices (no slow partition-
            # shift DMAs): auxU[p] = s[p+1, col], auxD[p] = s[p-1, col].
            pu = ps.tile([P, 2], F32, tag="shpsum")
            nc.tensor.matmul(pu[:, :], shiftu[:, :], s[:, 0:2])
            # partition 127 was pre-filled with BIG via DMA (engines can't
            # address a 1-partition range at p=127); copy only 0..126 here
            nc.scalar.copy(auxU[0 : P - 1, :], pu[0 : P - 1, :])
            pd = ps.tile([P, 2], F32, tag="shpsum")
            nc.tensor.matmul(pd[:, :], shiftd[:, :], s[:, W - 2 : W])
            auxD = sp.tile([P, 2], F32)
            nc.scalar.copy(auxD[:, :], pd[:, :])
            dR = sp.tile([P, W + 1], F32)   # col c: R1 at idx p*W + c - 1
            d2 = sp.tile([P, W + 2], F32)   # col c: R2 at idx p*W + c - 2
            nc.vector.tensor_sub(dR[:, 1:W], s[:, 1:W], s[:, 0 : W - 1])
            nc.vector.tensor_sub(dR[:, W : W + 1], auxU[:, 0:1], s[:, W - 1 : W])
            nc.vector.tensor_sub(dR[:, 0:1], s[:, 0:1], auxD[:, 1:2])
            nc.vector.memset(dR[0:1, 0:1], BIG)
            nc.vector.tensor_sub(d2[:, 2:W], s[:, 2:W], s[:, 0 : W - 2])
            nc.vector.tensor_sub(d2[:, W : W + 1], auxU[:, 0:1], s[:, W - 2 : W - 1])
            nc.vector.tensor_sub(d2[:, W + 1 : W + 2], auxU[:, 1:2], s[:, W - 1 : W])
            nc.vector.tensor_sub(d2[:, 0:2], s[:, 0:2], auxD[:, 0:2])
            nc.vector.memset(d2[0:1, 0:2], BIG)

            ca = dR[:, 1 : W + 1]   # R1
            cb = dR[:, 0:W]         # L1
            cc = d2[:, 2 : W + 2]   # R2
            cd = d2[:, 0:W]         # L2
            ab_lo = sp.tile([P, W], F32)
            ab_hi = sp.tile([P, W], F32)
            cd_lo = sp.tile([P, W], F32)
            cd_hi = sp.tile([P, W], F32)
            nc.vector.tensor_tensor(ab_lo[:, :], ca, cb, op=ALU.min)
            nc.vector.tensor_tensor(ab_hi[:, :], ca, cb, op=ALU.max)
            nc.vector.tensor_tensor(cd_lo[:, :], cc, cd, op=ALU.min)
            nc.vector.tensor_tensor(cd_hi[:, :], cc, cd, op=ALU.max)
            m1 = sp.tile([P, W], F32)
            mm = sp.tile([P, W], F32)
            nc.vector.tensor_tensor(m1[:, :], ab_lo[:, :], cd_lo[:, :], op=ALU.min)
            nc.vector.tensor_tensor(mm[:, :], ab_lo[:, :], cd_lo[:, :], op=ALU.max)
            nc.vector.tensor_tensor(ab_lo[:, :], ab_hi[:, :], cd_hi[:, :], op=ALU.min)
            nc.vector.tensor_tensor(mm[:, :], mm[:, :], ab_lo[:, :], op=ALU.min)
            # ship d1 + d2; the reciprocal/density happens on host
            nc.vector.tensor_add(out_sb[:, 0:W], m1[:, :], mm[:, :])

            nc.sync.dma_start(out[:, :], out_sb[:, :])
    nc.finalize()
    return nc


_NC_CACHE = None


def _get_nc():
    global _NC_CACHE
    if _NC_CACHE is None:
        _NC_CACHE = _build()
    return _NC_CACHE


def make_in_maps(x_input, targets, w1, b1, w2, b2):
    x_input = np.ascontiguousarray(x_input, dtype=np.float32)
    targets = np.ascontiguousarray(targets, dtype=np.float32)
    w1 = np.ascontiguousarray(w1, dtype=np.float32)
    b1 = np.ascontiguousarray(b1, dtype=np.float32)
    w2 = np.ascontiguousarray(w2, dtype=np.float32)
    b2 = np.ascontiguousarray(b2, dtype=np.float32)
    c2n = (2.0 * w1.astype(np.float64) ** 2 * w2.astype(np.float64)).astype(np.float32)
    pidx = np.arange(P)

    def signs_col(k):
        return np.where((pidx & (k // W)) == 0, 1.0, -1.0).astype(np.float32)

    signs = np.stack([signs_col(k) for k in SGN_KS], axis=1).astype(np.float32)
    identity = np.eye(P, dtype=np.float32)
    shiftu = np.eye(P, P, -1, dtype=np.float32)  # auxU[m] = s[m+1]
    shiftd = np.eye(P, P, 1, dtype=np.float32)   # auxD[m] = s[m-1]
    in_maps = []
    ones_row = np.ones(SHARD, np.float32)
    for c in range(NCORES):
        ts = targets[c * SHARD : (c + 1) * SHARD].reshape(NCH, P).T
        xflat = np.concatenate([x_input[c * SHARD : (c + 1) * SHARD], ones_row])[None, :]
        in_maps.append(
            {
                "x_full": x_input,
                "t_shard": np.ascontiguousarray(ts),
                "w1": w1,
                "b1": b1,
                "w2": w2,
                "b2": b2,
                "c2n": c2n,
                "x_flat": np.ascontiguousarray(xflat),
                "signs": signs,
                "ident": identity,
                "shiftu": shiftu,
                "shiftd": shiftd,
            }
        )
    return in_maps


def kernel(x_input, targets, w1, b1, w2, b2, **_ignored):
    in_maps = make_in_maps(x_input, targets, w1, b1, w2, b2)
    nc = _get_nc()
    res = run_bass_kernel_spmd(nc, in_maps, core_ids=list(range(NCORES)))
    outs = [r["out"] for r in res.results]

    dsum = outs[0][:, :W].astype(np.float64).ravel()  # d1 + d2 per point
    dens = 1.0 / (dsum / 3.0 + 2.0 * EPS)
    sse = sum(o[:, W].astype(np.float64).sum() for o in outs)
    d2sq = sum(o[:, W + 1].astype(np.float64).sum() for o in outs)

    mse = sse / N
    mean_densn = (dens.sum() / N) / (dens.max() + EPS)
    penalty = 0.01 * (1.0 + 0.1 * mean_densn) * (d2sq / N)
    total = mse + penalty
    return np.array([total, mse, penalty], dtype=np.float32)



# revision 11
# speedup vs baseline: 1.0940x; 1.0940x over previous
"""AdaptiveCurvatureLoss on 8 TRN2 NeuronCores — sign-mask bitonic kNN.

Every core sorts the full x (16384 values as a [128, 128] tile) with a bitonic
network in "masked space": descending blocks hold negated values so every
substage is a plain full-width min/max pair on the DVE (2 ops instead of the
4-op asc/desc view split).  Sign-state transitions ride for free wherever
possible:
  - the initial M_2 scale is applied by the host,
  - small-phase transitions are single tensor_tensor mask multiplies
    (partition-broadcast mask rows),
  - big-phase transitions are folded into the PE transposes as signed
    diagonal matrices (out = in.T @ D costs the same as in.T @ I),
  - the first substage after each transpose reads the PSUM tile directly,
    eliminating all post-transpose ACT copies.
The MLP / second-derivative / MSE parts run in a transposed layout (hidden
units on partitions, 512 points per tile, two 64-wide h-blocks packed per
128 partitions): u = x*w1 + b1 in one K=3 fp32 matmul, tanh/square on ACT,
one DVE op for g = (1 - th^2)*th, then bf16 PE matmuls compute
e = pred + b2 - t and d2 directly in PSUM (targets folded into the matmul
accumulation) with ACT Square+accum producing the scalar partials.
Host epilogue (O(N) numpy): neighbour-gap window from the sorted array,
density mean/max, and the final three scalars.
"""

import sys

sys.path.insert(0, "/opt/trn_rl_repo")

import numpy as np

import concourse.mybir as mybir
from concourse import bacc
from concourse.bass_utils import run_bass_kernel_spmd
from concourse.tile import TileContext

N = 16384
NCORES = 8
SHARD = N // NCORES          # 2048
P = 128
W = 128
H = 64
EPS = 1e-8
BIG = 1e30
F32 = mybir.dt.float32
BF16 = mybir.dt.bfloat16
ALU = mybir.AluOpType
ACTF = mybir.ActivationFunctionType

SMALL_KS = [2, 4, 8, 16, 32, 64]
BIG_KS = [256, 512, 1024, 2048, 4096, 8192, 16384]
NT = SHARD // 512            # 4 point-chunks of 512 -> 2 packed tiles
NTILE = 2                    # packed [128, 512] MLP tiles per core


def _build():
    nc = bacc.Bacc()
    xs = nc.declare_dram_parameter("xs", [P, W], F32, isOutput=False)
    cm = nc.declare_dram_parameter("cm", [1, 6 * W], F32, isOutput=False)
    pc = nc.declare_dram_parameter("pc", [P, 8], F32, isOutput=False)
    dg = nc.declare_dram_parameter("dg", [P, W], F32, isOutput=False)
    xmw = nc.declare_dram_parameter("xmw", [3, NTILE * 512 + P], F32, isOutput=False)
    wpc = nc.declare_dram_parameter("wpc", [P, 4], BF16, isOutput=False)
    trp = nc.declare_dram_parameter("trp", [3, NTILE * 512 + 2], BF16, isOutput=False)
    out_s = nc.declare_dram_parameter("out_s", [P, W], F32, isOutput=True)
    out_sums = nc.declare_dram_parameter("out_sums", [2, 2 * NTILE], F32, isOutput=True)

    with TileContext(nc) as tc:
        with (
            tc.tile_pool(name="sp", bufs=1) as sp,
            tc.tile_pool(name="tp", bufs=3, space="PSUM") as tp,
            tc.tile_pool(name="mu", bufs=2, space="PSUM") as mup,
            tc.tile_pool(name="ms", bufs=3, space="PSUM") as msp,
        ):
            # ---- loads ----
            sortA = sp.tile([P, W], F32)
            nc.sync.dma_start(sortA[:, :], xs[:, :])
            sortB = sp.tile([P, W], F32)
            cmt = sp.tile([P, 6 * W], F32)
            nc.gpsimd.dma_start(cmt[:, :], cm.ap().partition_broadcast(P))
            pct = sp.tile([P, 8], F32)
            nc.gpsimd.dma_start(pct[:, :], pc[:, :])
            dgt = sp.tile([P, W], F32)
            nc.gpsimd.dma_start(dgt[:, :], dg[:, :])
            xmt = sp.tile([3, NTILE * 512 + P], F32)
            nc.scalar.dma_start(xmt[:, :], xmw[:, :])
            wpt = sp.tile([P, 4], BF16)
            nc.scalar.dma_start(wpt[:, :], wpc[:, :])
            trt = sp.tile([3, NTILE * 512 + 2], BF16)
            nc.scalar.dma_start(trt[:, :], trp[:, :])

            th = sp.tile([P, NTILE, 512], BF16)
            th2 = sp.tile([P, NTILE, 512], BF16)
            g = sp.tile([P, NTILE, 512], BF16)
            esc = sp.tile([2, 512], BF16)
            sums = sp.tile([2, 2 * NTILE], F32)

            # ---- MLP emitters (interleaved into the sort) ----
            def emit_u_tanh(t):
                ups = mup.tile([P, 512], F32, tag="mu")
                nc.tensor.matmul(
                    ups[:, :], xmt[:, NTILE * 512 : NTILE * 512 + P],
                    xmt[:, t * 512 : (t + 1) * 512], start=True, stop=True,
                )
                nc.scalar.activation(th[:, t, :], ups[:, :], ACTF.Tanh)

            def emit_sq(t):
                nc.scalar.activation(th2[:, t, :], th[:, t, :], ACTF.Square)

            def emit_g(t):
                nc.vector.scalar_tensor_tensor(
                    g[:, t, :], th2[:, t, :], 1.0, th[:, t, :],
                    op0=ALU.subtract, op1=ALU.mult,
                )

            def emit_e_sse(t):
                pe = msp.tile([2, 512], F32, tag="ms")
                nc.tensor.matmul(
                    pe[:, :], wpt[:, 0:2], th[:, t, :], start=True, stop=False
                )
                nc.tensor.matmul(
                    pe[:, :], trt[:, NTILE * 512 : NTILE * 512 + 2],
                    trt[:, t * 512 : (t + 1) * 512], start=False, stop=True,
                )
                nc.scalar.activation(
                    esc[:, :], pe[:, :], ACTF.Square,
                    accum_out=sums[:, 2 * t : 2 * t + 1],
                )

            def emit_d2_sq(t):
                pd = msp.tile([2, 512], F32, tag="ms")
                nc.tensor.matmul(pd[:, :], wpt[:, 2:4], g[:, t, :], start=True, stop=True)
                nc.scalar.activation(
                    esc[:, :], pd[:, :], ACTF.Square,
                    accum_out=sums[:, 2 * t + 1 : 2 * t + 2],
                )

            # ---- sort helpers ----
            cur, alt = sortA, sortB

            def lohi(t, j):
                v = t[:, :].rearrange("p (c s) -> p c s", s=2 * j)
                return v[:, :, 0:j], v[:, :, j : 2 * j]

            def substage(j):
                nonlocal cur, alt
                lo, hi = lohi(cur, j)
                alo, ahi = lohi(alt, j)
                nc.vector.tensor_tensor(alo, lo, hi, op=ALU.min)
                nc.vector.tensor_tensor(ahi, lo, hi, op=ALU.max)
                cur, alt = alt, cur

            def mask(i):
                nonlocal cur, alt
                nc.vector.tensor_tensor(
                    alt[:, :], cur[:, :], cmt[:, i * W : (i + 1) * W], op=ALU.mult
                )
                cur, alt = alt, cur

            def pscale(i):
                nonlocal cur, alt
                nc.vector.tensor_scalar(
                    alt[:, :], cur[:, :], pct[:, i : i + 1], None, op0=ALU.mult
                )
                cur, alt = alt, cur

            # ---- phases 2..64 (masked space, host pre-applied M_2) ----
            emit_u_tanh(0)
            for ki, k in enumerate(SMALL_KS):
                j = k // 2
                while j >= 1:
                    substage(j)
                    j //= 2
                if k == 2:
                    emit_u_tanh(1)
                if k == 4:
                    emit_sq(0)
                if k == 8:
                    emit_sq(1)
                if k < 64:
                    mask(ki)
            # 64 -> 128 transition: f-part (M_64) then p-part (M_128)
            mask(5)
            pscale(0)
            # phase 128
            j = 64
            while j >= 1:
                substage(j)
                j //= 2

            # ---- phases 256..16384 ----
            for bi, k in enumerate(BIG_KS):
                # state change M_{k/2} -> M_k (per-p): DVE tensor_scalar
                pscale(bi + 1)
                pt = tp.tile([P, W], F32, tag="tpsum")
                nc.tensor.transpose(pt[:, :], cur[:, :], dgt[:, :])
                if bi == 0:
                    emit_g(0)
                    emit_g(1)
                if bi == 1:
                    emit_e_sse(0)
                if bi == 2:
                    emit_e_sse(1)
                if bi == 3:
                    emit_d2_sq(0)
                if bi == 4:
                    emit_d2_sq(1)
                if bi == 5:
                    nc.sync.dma_start(out_sums[:, :], sums[:, :])
                nc.scalar.copy(alt[:, :], pt[:, :])
                cur, alt = alt, cur
                jp = (k // W) // 2
                while jp >= 1:
                    substage(jp)
                    jp //= 2
                # exit transpose (plain identity)
                pt2 = tp.tile([P, W], F32, tag="tpsum")
                nc.tensor.transpose(pt2[:, :], cur[:, :], dgt[:, 0:W])
                nc.scalar.copy(alt[:, :], pt2[:, :])
                cur, alt = alt, cur
                j = 64
                while j >= 1:
                    substage(j)
                    j //= 2

            nc.sync.dma_start(out_s[:, :], cur[:, :])
    nc.finalize()
    return nc


_NC_CACHE = None


def _get_nc():
    global _NC_CACHE
    if _NC_CACHE is None:
        _NC_CACHE = _build()
    return _NC_CACHE


def _to_bf16(a):
    import ml_dtypes

    return np.ascontiguousarray(np.asarray(a, dtype=np.float32).astype(ml_dtypes.bfloat16))


def _msk(k):
    idx = np.arange(N).reshape(P, W)
    return np.where((idx & k) == 0, 1.0, -1.0).astype(np.float32)


def make_in_maps(x_input, targets, w1, b1, w2, b2):
    x_input = np.ascontiguousarray(x_input, dtype=np.float32)
    targets = np.ascontiguousarray(targets, dtype=np.float32)
    w1 = np.ascontiguousarray(w1, dtype=np.float32)
    b1 = np.ascontiguousarray(b1, dtype=np.float32)
    w2 = np.ascontiguousarray(w2, dtype=np.float32)
    b2 = np.ascontiguousarray(b2, dtype=np.float32)

    xs = (x_input.reshape(P, W) * _msk(2)).astype(np.float32)
    cm = np.concatenate(
        [(_msk(k) * _msk(2 * k))[0] for k in [2, 4, 8, 16, 32]] + [_msk(64)[0]]
    )[None, :].astype(np.float32)
    pidx = np.arange(P)

    def mp(k):  # M_k as a function of p, for k >= 128
        return np.where((pidx & (k // W)) == 0, 1.0, -1.0).astype(np.float32)

    # pc col 0 = M_128 (64->128 transition); col 1+bi = M_{k/2}*M_k per big k
    pcs = [mp(128)] + [mp(k // 2) * mp(k) for k in BIG_KS]
    pc = np.stack(pcs, axis=1).astype(np.float32)
    dg = np.eye(P, dtype=np.float32)

    # MLP packs (shared): u-matmul lhsT [3, 128] = [w1|0, 0|w1, b1|b1]
    wu = np.zeros((3, P), np.float32)
    wu[0, :H] = w1
    wu[1, H:] = w1
    wu[2, :H] = b1
    wu[2, H:] = b1
    # pred lhsT [128, 0:2] block-diag w2; d2 lhsT [128, 2:4] block-diag -c2
    c2n = (2.0 * w1.astype(np.float64) ** 2 * w2.astype(np.float64)).astype(np.float32)
    wp = np.zeros((P, 4), np.float32)
    wp[:H, 0] = w2
    wp[H:, 1] = w2
    wp[:H, 2] = -c2n
    wp[H:, 3] = -c2n
    wp = _to_bf16(wp)
    # e-matmul lhsT [3, 2] = [[1,0],[0,1],[b2,b2]]
    tp3 = np.array([[1.0, 0.0], [0.0, 1.0], [b2[0], b2[0]]], np.float32)

    in_maps = []
    for c in range(NCORES):
        xsh = x_input[c * SHARD : (c + 1) * SHARD]
        tsh = targets[c * SHARD : (c + 1) * SHARD]
        xm = np.zeros((3, NTILE * 512 + P), np.float32)
        tr = np.zeros((3, NTILE * 512 + 2), np.float32)
        for t in range(NTILE):
            xm[0, t * 512 : (t + 1) * 512] = xsh[t * 1024 : t * 1024 + 512]
            xm[1, t * 512 : (t + 1) * 512] = xsh[t * 1024 + 512 : (t + 1) * 1024]
            tr[0, t * 512 : (t + 1) * 512] = -tsh[t * 1024 : t * 1024 + 512]
            tr[1, t * 512 : (t + 1) * 512] = -tsh[t * 1024 + 512 : (t + 1) * 1024]
        xm[2, : NTILE * 512] = 1.0
        xm[:, NTILE * 512 :] = wu
        tr[2, : NTILE * 512] = 1.0
        tr[:, NTILE * 512 :] = tp3
        in_maps.append(
            {
                "xs": xs,
                "cm": cm,
                "pc": pc,
                "dg": np.ascontiguousarray(dg),
                "xmw": np.ascontiguousarray(xm),
                "wpc": wp,
                "trp": _to_bf16(tr),
            }
        )
    return in_maps


def kernel(x_input, targets, w1, b1, w2, b2, **_ignored):
    in_maps = make_in_maps(x_input, targets, w1, b1, w2, b2)
    nc = _get_nc()
    res = run_bass_kernel_spmd(nc, in_maps, core_ids=list(range(NCORES)))

    s = res.results[0]["out_s"].astype(np.float64).ravel()  # sorted ascending
    gp = np.diff(s)
    L1 = np.concatenate([[BIG], gp])
    R1 = np.concatenate([gp, [BIG]])
    gs = gp[:-1] + gp[1:]
    L2 = np.concatenate([[BIG, BIG], gs])
    R2 = np.concatenate([gs, [BIG, BIG]])
    d12 = np.minimum(np.minimum(L1 + R1, L1 + L2), R1 + R2)
    dens = 1.0 / (d12 / 3.0 + 2.0 * EPS)
    m = (dens.sum() / N) / (dens.max() + EPS)

    sse = sum(r["out_sums"].astype(np.float64).sum(axis=0)[0::2].sum() for r in res.results)
    d2sq = sum(r["out_sums"].astype(np.float64).sum(axis=0)[1::2].sum() for r in res.results)

    mse = sse / N
    penalty = 0.01 * (1.0 + 0.1 * m) * (d2sq / N)
    total = mse + penalty
    return np.array([total, mse, penalty], dtype=np.float32)


# revision 13
# speedup vs baseline: 1.0981x; 1.0038x over previous
"""AdaptiveCurvatureLoss on 8 TRN2 NeuronCores — sign-mask bitonic kNN.

Every core sorts the full x (16384 values as a [128, 128] tile) with a bitonic
network in "masked space": descending blocks hold negated values so every
substage is a plain full-width min/max pair on the DVE (2 ops instead of the
4-op asc/desc view split).  Sign-state transitions ride for free wherever
possible:
  - the initial M_2 scale is applied by the host,
  - small-phase transitions are single tensor_tensor mask multiplies
    (partition-broadcast mask rows),
  - big-phase transitions are folded into the PE transposes as signed
    diagonal matrices (out = in.T @ D costs the same as in.T @ I),
  - the first substage after each transpose reads the PSUM tile directly,
    eliminating all post-transpose ACT copies.
The MLP / second-derivative / MSE parts run in a transposed layout (hidden
units on partitions, 512 points per tile, two 64-wide h-blocks packed per
128 partitions): u = x*w1 + b1 in one K=3 fp32 matmul, tanh/square on ACT,
one DVE op for g = (1 - th^2)*th, then bf16 PE matmuls compute
e = pred + b2 - t and d2 directly in PSUM (targets folded into the matmul
accumulation) with ACT Square+accum producing the scalar partials.
Host epilogue (O(N) numpy): neighbour-gap window from the sorted array,
density mean/max, and the final three scalars.
"""

import sys

sys.path.insert(0, "/opt/trn_rl_repo")

import numpy as np

import concourse.mybir as mybir
from concourse import bacc
from concourse.bass_utils import run_bass_kernel_spmd
from concourse.tile import TileContext

N = 16384
NCORES = 8
SHARD = N // NCORES          # 2048
P = 128
W = 128
H = 64
EPS = 1e-8
BIG = 1e30
F32 = mybir.dt.float32
BF16 = mybir.dt.bfloat16
ALU = mybir.AluOpType
ACTF = mybir.ActivationFunctionType

SMALL_KS = [2, 4, 8, 16, 32, 64]
BIG_KS = [256, 512, 1024, 2048, 4096, 8192, 16384]
NT = SHARD // 512            # 4 point-chunks of 512 -> 2 packed tiles
NTILE = 2                    # packed [128, 512] MLP tiles per core


def _build():
    nc = bacc.Bacc()
    xs = nc.declare_dram_parameter("xs", [P, W], F32, isOutput=False)
    cm = nc.declare_dram_parameter("cm", [1, 6 * W], F32, isOutput=False)
    pc = nc.declare_dram_parameter("pc", [P, 8], F32, isOutput=False)
    dg = nc.declare_dram_parameter("dg", [P, W], F32, isOutput=False)
    xmw = nc.declare_dram_parameter("xmw", [3, NTILE * 512 + P], F32, isOutput=False)
    wpc = nc.declare_dram_parameter("wpc", [P, 4], BF16, isOutput=False)
    trp = nc.declare_dram_parameter("trp", [3, NTILE * 512 + 2], BF16, isOutput=False)
    out_s = nc.declare_dram_parameter("out_s", [P, W], F32, isOutput=True)
    out_sums = nc.declare_dram_parameter("out_sums", [2, 2 * NTILE], F32, isOutput=True)

    with TileContext(nc) as tc:
        with (
            tc.tile_pool(name="sp", bufs=1) as sp,
            tc.tile_pool(name="tp", bufs=3, space="PSUM") as tp,
            tc.tile_pool(name="mu", bufs=2, space="PSUM") as mup,
            tc.tile_pool(name="ms", bufs=3, space="PSUM") as msp,
        ):
            # ---- loads ----
            sortA = sp.tile([P, W], F32)
            nc.sync.dma_start(sortA[:, :], xs[:, :])
            sortB = sp.tile([P, W], F32)
            cmt = sp.tile([P, 6 * W], F32)
            nc.gpsimd.dma_start(cmt[:, :], cm.ap().partition_broadcast(P))
            pct = sp.tile([P, 8], F32)
            nc.gpsimd.dma_start(pct[:, :], pc[:, :])
            dgt = sp.tile([P, W], F32)
            nc.gpsimd.dma_start(dgt[:, :], dg[:, :])
            xmt = sp.tile([3, NTILE * 512 + P], F32)
            nc.scalar.dma_start(xmt[:, :], xmw[:, :])
            wpt = sp.tile([P, 4], BF16)
            nc.scalar.dma_start(wpt[:, :], wpc[:, :])
            trt = sp.tile([3, NTILE * 512 + 2], BF16)
            nc.scalar.dma_start(trt[:, :], trp[:, :])

            th = sp.tile([P, NTILE, 512], BF16)
            th2 = sp.tile([P, NTILE, 512], BF16)
            g = sp.tile([P, NTILE, 512], BF16)
            esc = sp.tile([2, 512], BF16)
            sums = sp.tile([2, 2 * NTILE], F32)

            # ---- MLP emitters (interleaved into the sort) ----
            def emit_u_tanh(t):
                ups = mup.tile([P, 512], F32, tag="mu")
                nc.tensor.matmul(
                    ups[:, :], xmt[:, NTILE * 512 : NTILE * 512 + P],
                    xmt[:, t * 512 : (t + 1) * 512], start=True, stop=True,
                )
                nc.scalar.activation(th[:, t, :], ups[:, :], ACTF.Tanh)

            def emit_sq(t):
                nc.scalar.activation(th2[:, t, :], th[:, t, :], ACTF.Square)

            def emit_g(t):
                nc.vector.scalar_tensor_tensor(
                    g[:, t, :], th2[:, t, :], 1.0, th[:, t, :],
                    op0=ALU.subtract, op1=ALU.mult,
                )

            def emit_e_sse(t):
                pe = msp.tile([2, 512], F32, tag="ms")
                nc.tensor.matmul(
                    pe[:, :], wpt[:, 0:2], th[:, t, :], start=True, stop=False
                )
                nc.tensor.matmul(
                    pe[:, :], trt[:, NTILE * 512 : NTILE * 512 + 2],
                    trt[:, t * 512 : (t + 1) * 512], start=False, stop=True,
                )
                nc.scalar.activation(
                    esc[:, :], pe[:, :], ACTF.Square,
                    accum_out=sums[:, 2 * t : 2 * t + 1],
                )

            def emit_d2_sq(t):
                pd = msp.tile([2, 512], F32, tag="ms")
                nc.tensor.matmul(pd[:, :], wpt[:, 2:4], g[:, t, :], start=True, stop=True)
                nc.scalar.activation(
                    esc[:, :], pd[:, :], ACTF.Square,
                    accum_out=sums[:, 2 * t + 1 : 2 * t + 2],
                )

            # ---- sort helpers ----
            cur, alt = sortA, sortB

            def lohi(t, j):
                v = t[:, :].rearrange("p (c s) -> p c s", s=2 * j)
                return v[:, :, 0:j], v[:, :, j : 2 * j]

            def substage(j):
                nonlocal cur, alt
                lo, hi = lohi(cur, j)
                alo, ahi = lohi(alt, j)
                nc.vector.tensor_tensor(alo, lo, hi, op=ALU.min)
                nc.vector.tensor_tensor(ahi, lo, hi, op=ALU.max)
                cur, alt = alt, cur

            def mask(i):
                nonlocal cur, alt
                nc.vector.tensor_tensor(
                    alt[:, :], cur[:, :], cmt[:, i * W : (i + 1) * W], op=ALU.mult
                )
                cur, alt = alt, cur

            def pscale(i):
                nonlocal cur, alt
                nc.vector.tensor_scalar(
                    alt[:, :], cur[:, :], pct[:, i : i + 1], None, op0=ALU.mult
                )
                cur, alt = alt, cur

            # ---- phases 2..64 (masked space, host pre-applied M_2) ----
            emit_u_tanh(0)
            for ki, k in enumerate(SMALL_KS):
                j = k // 2
                while j >= 1:
                    substage(j)
                    j //= 2
                if k == 2:
                    emit_u_tanh(1)
                if k == 4:
                    emit_sq(0)
                if k == 8:
                    emit_sq(1)
                if k < 64:
                    mask(ki)
            # 64 -> 128 transition: f-part (M_64) then p-part (M_128)
            mask(5)
            pscale(0)
            # phase 128
            j = 64
            while j >= 1:
                substage(j)
                j //= 2

            # ---- phases 256..16384 ----
            for bi, k in enumerate(BIG_KS):
                # state change M_{k/2} -> M_k (per-p): DVE tensor_scalar
                pscale(bi + 1)
                pt = tp.tile([P, W], F32, tag="tpsum")
                nc.tensor.transpose(pt[:, :], cur[:, :], dgt[:, :])
                if bi == 0:
                    emit_g(0)
                    emit_g(1)
                if bi == 1:
                    emit_e_sse(0)
                if bi == 2:
                    emit_e_sse(1)
                if bi == 3:
                    emit_d2_sq(0)
                if bi == 4:
                    emit_d2_sq(1)
                if bi == 5:
                    nc.sync.dma_start(out_sums[:, :], sums[:, :])
                nc.vector.tensor_copy(alt[:, :], pt[:, :])
                cur, alt = alt, cur
                jp = (k // W) // 2
                while jp >= 1:
                    substage(jp)
                    jp //= 2
                # exit transpose (plain identity)
                pt2 = tp.tile([P, W], F32, tag="tpsum")
                nc.tensor.transpose(pt2[:, :], cur[:, :], dgt[:, 0:W])
                nc.vector.tensor_copy(alt[:, :], pt2[:, :])
                cur, alt = alt, cur
                j = 64
                while j >= 1:
                    substage(j)
                    j //= 2

            nc.sync.dma_start(out_s[0 : P // 2, :], cur[0 : P // 2, :])
            nc.gpsimd.dma_start(out_s[P // 2 : P, :], cur[P // 2 : P, :])
    nc.finalize()
    return nc


_NC_CACHE = None


def _get_nc():
    global _NC_CACHE
    if _NC_CACHE is None:
        _NC_CACHE = _build()
    return _NC_CACHE


def _to_bf16(a):
    import ml_dtypes

    return np.ascontiguousarray(np.asarray(a, dtype=np.float32).astype(ml_dtypes.bfloat16))


def _msk(k):
    idx = np.arange(N).reshape(P, W)
    return np.where((idx & k) == 0, 1.0, -1.0).astype(np.float32)


def make_in_maps(x_input, targets, w1, b1, w2, b2):
    x_input = np.ascontiguousarray(x_input, dtype=np.float32)
    targets = np.ascontiguousarray(targets, dtype=np.float32)
    w1 = np.ascontiguousarray(w1, dtype=np.float32)
    b1 = np.ascontiguousarray(b1, dtype=np.float32)
    w2 = np.ascontiguousarray(w2, dtype=np.float32)
    b2 = np.ascontiguousarray(b2, dtype=np.float32)

    xs = (x_input.reshape(P, W) * _msk(2)).astype(np.float32)
    cm = np.concatenate(
        [(_msk(k) * _msk(2 * k))[0] for k in [2, 4, 8, 16, 32]] + [_msk(64)[0]]
    )[None, :].astype(np.float32)
    pidx = np.arange(P)

    def mp(k):  # M_k as a function of p, for k >= 128
        return np.where((pidx & (k // W)) == 0, 1.0, -1.0).astype(np.float32)

    # pc col 0 = M_128 (64->128 transition); col 1+bi = M_{k/2}*M_k per big k
    pcs = [mp(128)] + [mp(k // 2) * mp(k) for k in BIG_KS]
    pc = np.stack(pcs, axis=1).astype(np.float32)
    dg = np.eye(P, dtype=np.float32)

    # MLP packs (shared): u-matmul lhsT [3, 128] = [w1|0, 0|w1, b1|b1]
    wu = np.zeros((3, P), np.float32)
    wu[0, :H] = w1
    wu[1, H:] = w1
    wu[2, :H] = b1
    wu[2, H:] = b1
    # pred lhsT [128, 0:2] block-diag w2; d2 lhsT [128, 2:4] block-diag -c2
    c2n = (2.0 * w1.astype(np.float64) ** 2 * w2.astype(np.float64)).astype(np.float32)
    wp = np.zeros((P, 4), np.float32)
    wp[:H, 0] = w2
    wp[H:, 1] = w2
    wp[:H, 2] = -c2n
    wp[H:, 3] = -c2n
    wp = _to_bf16(wp)
    # e-matmul lhsT [3, 2] = [[1,0],[0,1],[b2,b2]]
    tp3 = np.array([[1.0, 0.0], [0.0, 1.0], [b2[0], b2[0]]], np.float32)

    in_maps = []
    for c in range(NCORES):
        xsh = x_input[c * SHARD : (c + 1) * SHARD]
        tsh = targets[c * SHARD : (c + 1) * SHARD]
        xm = np.zeros((3, NTILE * 512 + P), np.float32)
        tr = np.zeros((3, NTILE * 512 + 2), np.float32)
        for t in range(NTILE):
            xm[0, t * 512 : (t + 1) * 512] = xsh[t * 1024 : t * 1024 + 512]
            xm[1, t * 512 : (t + 1) * 512] = xsh[t * 1024 + 512 : (t + 1) * 1024]
            tr[0, t * 512 : (t + 1) * 512] = -tsh[t * 1024 : t * 1024 + 512]
            tr[1, t * 512 : (t + 1) * 512] = -tsh[t * 1024 + 512 : (t + 1) * 1024]
        xm[2, : NTILE * 512] = 1.0
        xm[:, NTILE * 512 :] = wu
        tr[2, : NTILE * 512] = 1.0
        tr[:, NTILE * 512 :] = tp3
        in_maps.append(
            {
                "xs": xs,
                "cm": cm,
                "pc": pc,
                "dg": np.ascontiguousarray(dg),
                "xmw": np.ascontiguousarray(xm),
                "wpc": wp,
                "trp": _to_bf16(tr),
            }
        )
    return in_maps


def kernel(x_input, targets, w1, b1, w2, b2, **_ignored):
    in_maps = make_in_maps(x_input, targets, w1, b1, w2, b2)
    nc = _get_nc()
    res = run_bass_kernel_spmd(nc, in_maps, core_ids=list(range(NCORES)))

    s = res.results[0]["out_s"].astype(np.float64).ravel()  # sorted ascending
    gp = np.diff(s)
    L1 = np.concatenate([[BIG], gp])
    R1 = np.concatenate([gp, [BIG]])
    gs = gp[:-1] + gp[1:]
    L2 = np.concatenate([[BIG, BIG], gs])
    R2 = np.concatenate([gs, [BIG, BIG]])
    d12 = np.minimum(np.minimum(L1 + R1, L1 + L2), R1 + R2)
    dens = 1.0 / (d12 / 3.0 + 2.0 * EPS)
    m = (dens.sum() / N) / (dens.max() + EPS)

    sse = sum(r["out_sums"].astype(np.float64).sum(axis=0)[0::2].sum() for r in res.results)
    d2sq = sum(r["out_sums"].astype(np.float64).sum(axis=0)[1::2].sum() for r in res.results)

    mse = sse / N
    penalty = 0.01 * (1.0 + 0.1 * m) * (d2sq / N)
    total = mse + penalty
    return np.array([total, mse, penalty], dtype=np.float32)


# revision 18
# speedup vs baseline: 1.1269x; 1.0262x over previous
"""AdaptiveCurvatureLoss on 8 TRN2 NeuronCores — sign-mask bitonic kNN.

Every core sorts the full x (16384 values as a [128, 128] tile) with a bitonic
network in "masked space": descending blocks hold negated values so every
substage is a plain full-width min/max pair on the DVE (2 ops instead of the
4-op asc/desc view split).  Sign-state transitions ride for free wherever
possible:
  - the initial M_2 scale is applied by the host,
  - small-phase transitions are single tensor_tensor mask multiplies
    (partition-broadcast mask rows),
  - big-phase transitions are folded into the PE transposes as signed
    diagonal matrices (out = in.T @ D costs the same as in.T @ I),
  - the first substage after each transpose reads the PSUM tile directly,
    eliminating all post-transpose ACT copies.
The MLP / second-derivative / MSE parts run in a transposed layout (hidden
units on partitions, 512 points per tile, two 64-wide h-blocks packed per
128 partitions): u = x*w1 + b1 in one K=3 fp32 matmul, tanh/square on ACT,
one DVE op for g = (1 - th^2)*th, then bf16 PE matmuls compute
e = pred + b2 - t and d2 directly in PSUM (targets folded into the matmul
accumulation) with ACT Square+accum producing the scalar partials.
Host epilogue (O(N) numpy): neighbour-gap window from the sorted array,
density mean/max, and the final three scalars.
"""

import sys

sys.path.insert(0, "/opt/trn_rl_repo")

import numpy as np

import concourse.mybir as mybir
from concourse import bacc
from concourse.bass_utils import run_bass_kernel_spmd
from concourse.tile import TileContext

N = 16384
NCORES = 8
SHARD = N // NCORES          # 2048
P = 128
W = 128
H = 64
EPS = 1e-8
BIG = 1e30
F32 = mybir.dt.float32
BF16 = mybir.dt.bfloat16
ALU = mybir.AluOpType
ACTF = mybir.ActivationFunctionType

SMALL_KS = [2, 4, 8, 16, 32, 64]
BIG_KS = [256, 512, 1024, 2048, 4096, 8192, 16384]
NT = SHARD // 512            # 4 point-chunks of 512 -> 2 packed tiles
NTILE = 2                    # packed [128, 512] MLP tiles per core


def _build():
    nc = bacc.Bacc()
    xs = nc.declare_dram_parameter("xs", [P, W], F32, isOutput=False)
    cm = nc.declare_dram_parameter("cm", [1, 13 * W], F32, isOutput=False)
    pc = nc.declare_dram_parameter("pc", [P, 8], F32, isOutput=False)
    dg = nc.declare_dram_parameter("dg", [P, W], F32, isOutput=False)
    xmw = nc.declare_dram_parameter("xmw", [3, NTILE * 512 + P], F32, isOutput=False)
    wpc = nc.declare_dram_parameter("wpc", [P, 4], BF16, isOutput=False)
    trp = nc.declare_dram_parameter("trp", [3, NTILE * 512 + 2], BF16, isOutput=False)
    out_s = nc.declare_dram_parameter("out_s", [P, W], F32, isOutput=True)
    out_sums = nc.declare_dram_parameter("out_sums", [2, 2 * NTILE], F32, isOutput=True)

    with TileContext(nc) as tc:
        with (
            tc.tile_pool(name="sp", bufs=1) as sp,
            tc.tile_pool(name="tp", bufs=3, space="PSUM") as tp,
            tc.tile_pool(name="mu", bufs=2, space="PSUM") as mup,
            tc.tile_pool(name="ms", bufs=3, space="PSUM") as msp,
        ):
            # ---- loads ----
            sortA = sp.tile([P, W], F32)
            nc.sync.dma_start(sortA[:, :], xs[:, :])
            sortB = sp.tile([P, W], F32)
            cmt = sp.tile([P, 13 * W], F32)
            nc.gpsimd.dma_start(cmt[:, :], cm.ap().partition_broadcast(P))
            pct = sp.tile([P, 8], F32)
            nc.gpsimd.dma_start(pct[:, :], pc[:, :])
            dgt = sp.tile([P, W], F32)
            nc.gpsimd.dma_start(dgt[:, :], dg[:, :])
            xmt = sp.tile([3, NTILE * 512 + P], F32)
            nc.scalar.dma_start(xmt[:, :], xmw[:, :])
            wpt = sp.tile([P, 4], BF16)
            nc.scalar.dma_start(wpt[:, :], wpc[:, :])
            trt = sp.tile([3, NTILE * 512 + 2], BF16)
            nc.scalar.dma_start(trt[:, :], trp[:, :])

            th = sp.tile([P, NTILE, 512], BF16)
            th2 = sp.tile([P, NTILE, 512], BF16)
            g = sp.tile([P, NTILE, 512], BF16)
            esc = sp.tile([2, 512], BF16)
            sums = sp.tile([2, 2 * NTILE], F32)

            # ---- MLP emitters (interleaved into the sort) ----
            def emit_u_tanh(t):
                ups = mup.tile([P, 512], F32, tag="mu")
                nc.tensor.matmul(
                    ups[:, :], xmt[:, NTILE * 512 : NTILE * 512 + P],
                    xmt[:, t * 512 : (t + 1) * 512], start=True, stop=True,
                )
                nc.scalar.activation(th[:, t, :], ups[:, :], ACTF.Tanh)

            def emit_sq(t):
                nc.scalar.activation(th2[:, t, :], th[:, t, :], ACTF.Square)

            def emit_g(t):
                nc.vector.scalar_tensor_tensor(
                    g[:, t, :], th2[:, t, :], 1.0, th[:, t, :],
                    op0=ALU.subtract, op1=ALU.mult,
                )

            def emit_e_sse(t):
                pe = msp.tile([2, 512], F32, tag="ms")
                nc.tensor.matmul(
                    pe[:, :], wpt[:, 0:2], th[:, t, :], start=True, stop=False
                )
                nc.tensor.matmul(
                    pe[:, :], trt[:, NTILE * 512 : NTILE * 512 + 2],
                    trt[:, t * 512 : (t + 1) * 512], start=False, stop=True,
                )
                nc.scalar.activation(
                    esc[:, :], pe[:, :], ACTF.Square,
                    accum_out=sums[:, 2 * t : 2 * t + 1],
                )

            def emit_d2_sq(t):
                pd = msp.tile([2, 512], F32, tag="ms")
                nc.tensor.matmul(pd[:, :], wpt[:, 2:4], g[:, t, :], start=True, stop=True)
                nc.scalar.activation(
                    esc[:, :], pd[:, :], ACTF.Square,
                    accum_out=sums[:, 2 * t + 1 : 2 * t + 2],
                )

            # ---- sort helpers ----
            cur, alt = sortA, sortB

            def lohi(t, j):
                v = t[:, :].rearrange("p (c s) -> p c s", s=2 * j)
                return v[:, :, 0:j], v[:, :, j : 2 * j]

            def substage(j):
                nonlocal cur, alt
                lo, hi = lohi(cur, j)
                alo, ahi = lohi(alt, j)
                nc.vector.tensor_tensor(alo, lo, hi, op=ALU.min)
                nc.vector.tensor_tensor(ahi, lo, hi, op=ALU.max)
                cur, alt = alt, cur

            def mask(i):
                nonlocal cur, alt
                nc.vector.tensor_tensor(
                    alt[:, :], cur[:, :], cmt[:, i * W : (i + 1) * W], op=ALU.mult
                )
                cur, alt = alt, cur

            def pscale(i):
                nonlocal cur, alt
                nc.vector.tensor_scalar(
                    alt[:, :], cur[:, :], pct[:, i : i + 1], None, op0=ALU.mult
                )
                cur, alt = alt, cur

            # ---- phases 2..64 (masked space, host pre-applied M_2) ----
            emit_u_tanh(0)
            for ki, k in enumerate(SMALL_KS):
                j = k // 2
                while j >= 1:
                    substage(j)
                    j //= 2
                if k == 2:
                    emit_u_tanh(1)
                if k == 4:
                    emit_sq(0)
                if k == 8:
                    emit_sq(1)
                if k < 64:
                    mask(ki)
            # 64 -> 128 transition: f-part (M_64) then p-part (M_128)
            mask(5)
            pscale(0)
            # phase 128
            j = 64
            while j >= 1:
                substage(j)
                j //= 2

            # ---- phases 256..16384 ----
            for bi, k in enumerate(BIG_KS):
                pt = tp.tile([P, W], F32, tag="tpsum")
                nc.tensor.transpose(pt[:, :], cur[:, :], dgt[:, :])
                if bi == 0:
                    emit_g(0)
                    emit_g(1)
                if bi == 1:
                    emit_e_sse(0)
                if bi == 2:
                    emit_e_sse(1)
                if bi == 3:
                    emit_d2_sq(0)
                if bi == 4:
                    emit_d2_sq(1)
                if bi == 5:
                    nc.sync.dma_start(out_sums[:, :], sums[:, :])
                # copy + state change M_{k/2} -> M_k (free-dim in T-space)
                nc.vector.tensor_tensor(
                    alt[:, :], pt[:, :], cmt[:, (6 + bi) * W : (7 + bi) * W],
                    op=ALU.mult,
                )
                cur, alt = alt, cur
                jp = (k // W) // 2
                while jp >= 1:
                    substage(jp)
                    jp //= 2
                # exit transpose (plain identity)
                pt2 = tp.tile([P, W], F32, tag="tpsum")
                nc.tensor.transpose(pt2[:, :], cur[:, :], dgt[:, 0:W])
                nc.vector.tensor_copy(alt[:, :], pt2[:, :])
                cur, alt = alt, cur
                j = 64
                while j >= 1:
                    substage(j)
                    j //= 2

            nc.sync.dma_start(out_s[0 : P // 2, :], cur[0 : P // 2, :])
            nc.gpsimd.dma_start(out_s[P // 2 : P, :], cur[P // 2 : P, :])
    nc.finalize()
    return nc


_NC_CACHE = None


def _get_nc():
    global _NC_CACHE
    if _NC_CACHE is None:
        _NC_CACHE = _build()
    return _NC_CACHE


def _to_bf16(a):
    import ml_dtypes

    return np.ascontiguousarray(np.asarray(a, dtype=np.float32).astype(ml_dtypes.bfloat16))


def _msk(k):
    idx = np.arange(N).reshape(P, W)
    return np.where((idx & k) == 0, 1.0, -1.0).astype(np.float32)


def make_in_maps(x_input, targets, w1, b1, w2, b2):
    x_input = np.ascontiguousarray(x_input, dtype=np.float32)
    targets = np.ascontiguousarray(targets, dtype=np.float32)
    w1 = np.ascontiguousarray(w1, dtype=np.float32)
    b1 = np.ascontiguousarray(b1, dtype=np.float32)
    w2 = np.ascontiguousarray(w2, dtype=np.float32)
    b2 = np.ascontiguousarray(b2, dtype=np.float32)

    xs = (x_input.reshape(P, W) * _msk(2)).astype(np.float32)
    pidx0 = np.arange(P)

    def mp0(k):
        return np.where((pidx0 & (k // W)) == 0, 1.0, -1.0).astype(np.float32)

    cm = np.concatenate(
        [(_msk(k) * _msk(2 * k))[0] for k in [2, 4, 8, 16, 32]]
        + [_msk(64)[0]]
        + [mp0(k // 2) * mp0(k) for k in BIG_KS]  # Delta_k along T-space free dim
    )[None, :].astype(np.float32)
    pidx = np.arange(P)

    def mp(k):  # M_k as a function of p, for k >= 128
        return np.where((pidx & (k // W)) == 0, 1.0, -1.0).astype(np.float32)

    # pc col 0 = M_128 (64->128 transition); col 1+bi = M_{k/2}*M_k per big k
    pcs = [mp(128)] + [mp(k // 2) * mp(k) for k in BIG_KS]
    pc = np.stack(pcs, axis=1).astype(np.float32)
    dg = np.eye(P, dtype=np.float32)

    # MLP packs (shared): u-matmul lhsT [3, 128] = [w1|0, 0|w1, b1|b1]
    wu = np.zeros((3, P), np.float32)
    wu[0, :H] = w1
    wu[1, H:] = w1
    wu[2, :H] = b1
    wu[2, H:] = b1
    # pred lhsT [128, 0:2] block-diag w2; d2 lhsT [128, 2:4] block-diag -c2
    c2n = (2.0 * w1.astype(np.float64) ** 2 * w2.astype(np.float64)).astype(np.float32)
    wp = np.zeros((P, 4), np.float32)
    wp[:H, 0] = w2
    wp[H:, 1] = w2
    wp[:H, 2] = -c2n
    wp[H:, 3] = -c2n
    wp = _to_bf16(wp)
    # e-matmul lhsT [3, 2] = [[1,0],[0,1],[b2,b2]]
    tp3 = np.array([[1.0, 0.0], [0.0, 1.0], [b2[0], b2[0]]], np.float32)

    in_maps = []
    for c in range(NCORES):
        xsh = x_input[c * SHARD : (c + 1) * SHARD]
        tsh = targets[c * SHARD : (c + 1) * SHARD]
        xm = np.zeros((3, NTILE * 512 + P), np.float32)
        tr = np.zeros((3, NTILE * 512 + 2), np.float32)
        for t in range(NTILE):
            xm[0, t * 512 : (t + 1) * 512] = xsh[t * 1024 : t * 1024 + 512]
            xm[1, t * 512 : (t + 1) * 512] = xsh[t * 1024 + 512 : (t + 1) * 1024]
            tr[0, t * 512 : (t + 1) * 512] = -tsh[t * 1024 : t * 1024 + 512]
            tr[1, t * 512 : (t + 1) * 512] = -tsh[t * 1024 + 512 : (t + 1) * 1024]
        xm[2, : NTILE * 512] = 1.0
        xm[:, NTILE * 512 :] = wu
        tr[2, : NTILE * 512] = 1.0
        tr[:, NTILE * 512 :] = tp3
        in_maps.append(
            {
                "xs": xs,
                "cm": cm,
                "pc": pc,
                "dg": np.ascontiguousarray(dg),
                "xmw": np.ascontiguousarray(xm),
                "wpc": wp,
                "trp": _to_bf16(tr),
            }
        )
    return in_maps


def kernel(x_input, targets, w1, b1, w2, b2, **_ignored):
    in_maps = make_in_maps(x_input, targets, w1, b1, w2, b2)
    nc = _get_nc()
    res = run_bass_kernel_spmd(nc, in_maps, core_ids=list(range(NCORES)))

    s = res.results[0]["out_s"].astype(np.float64).ravel()  # sorted ascending
    gp = np.diff(s)
    L1 = np.concatenate([[BIG], gp])
    R1 = np.concatenate([gp, [BIG]])
    gs = gp[:-1] + gp[1:]
    L2 = np.concatenate([[BIG, BIG], gs])
    R2 = np.concatenate([gs, [BIG, BIG]])
    d12 = np.minimum(np.minimum(L1 + R1, L1 + L2), R1 + R2)
    dens = 1.0 / (d12 / 3.0 + 2.0 * EPS)
    m = (dens.sum() / N) / (dens.max() + EPS)

    sse = sum(r["out_sums"].astype(np.float64).sum(axis=0)[0::2].sum() for r in res.results)
    d2sq = sum(r["out_sums"].astype(np.float64).sum(axis=0)[1::2].sum() for r in res.results)

    mse = sse / N
    penalty = 0.01 * (1.0 + 0.1 * m) * (d2sq / N)
    total = mse + penalty
    return np.array([total, mse, penalty], dtype=np.float32)
